# revision 6
# baseline (speedup 1.0000x reference)
"""Trainium2 Bass kernel for 16-head self-attention (D=1024, S=2048, B=2)
with upper-triangular (j >= i) mask and scale 1/head_dim.

Sharding: batch*head-group parallel over 8 cores. Core c handles batch
c//4, heads [4*(c%4), 4*(c%4)+4). Each core computes Q/K/V projections for
its 256 output dims, attention for its 4 heads, and a partial output
projection (its 256 rows of wo). Host sums the 4 partials per batch.

On-chip layout is transposed end-to-end: QT/KT [dh, seq], scores S^T
[seq_k, seq_q] (stationary=K^T chunk, moving=Q^T), exp on ScalarE
PSUM->SBUF with the 1/64 scale folded in, PV as O'^T = V'^T E^T with a
ones-column appended to V so row 64 of O' is the softmax denominator,
then out^T = wo^T O^T. The host transposes back.

Softmax normalization: the denominator row (row 64 of O') is broadcast
to 64 partitions with a K=1 ones matmul, then reciprocal runs on the
[64, 512] broadcast tile (all lanes busy) instead of on the raw [1, S]
row (single-lane DVE, ~13us per head).
"""

import sys

sys.path.insert(0, "/opt/trn_rl_repo")

import numpy as np

import concourse.bass as bass
import concourse.mybir as mybir
from concourse import tile
from concourse.bass_utils import run_bass_kernel_spmd

# ---------------------------------------------------------------------------
# Workaround: this walrus build supports only 1 sync wait on the SP CTRL
# (drain) instruction; split the TileContext exit drain's waits across
# sequential drains (same-engine program order makes this equivalent).
_MAX_DRAIN_WAITS = 1


def _patched_drain_and_barrier(self, tick_clock, wait_clock):
    from bass_rust import ScopedClock

    nc = self.nc
    drain_inst = nc.sync.drain()
    wait_clock.add_sem_waits(
        drain_inst.ins, ScopedClock({None: tick_clock.global_clock})
    )
    si = drain_inst.ins.sync_info
    if si is not None and len(si.on_wait) > _MAX_DRAIN_WAITS:
        waits = list(si.on_wait)
        si.on_wait = waits[:_MAX_DRAIN_WAITS]
        rest = waits[_MAX_DRAIN_WAITS:]
        while rest:
            chunk, rest = rest[:_MAX_DRAIN_WAITS], rest[_MAX_DRAIN_WAITS:]
            extra = nc.sync.drain()
            esi = extra.ins.sync_info
            if esi is None:
                extra.ins.sync_info = mybir.SyncInfo(on_wait=chunk, on_update=[])
            else:
                esi.on_wait = chunk
    nc.all_engine_barrier()
    assert self.sems is not None
    popped = nc._tile_sem_poison_stack.pop()
    assert popped is self._sem_poison
    nc.clear_and_free_semaphores(list(self.sems.allocated().values()))
    nc.all_engine_barrier()


tile.TileContext._drain_and_barrier = _patched_drain_and_barrier


def _legalize_waits(nc, max_waits=1):
    """This walrus build accepts at most one sync wait per instruction.
    Hoist extra waits onto preceding NoOps on the same engine (same-engine
    program order preserves the gating semantics)."""
    for blk in nc.main_func.blocks:
        out = []
        for inst in blk.instructions:
            si = inst.sync_info
            if si is not None and len(si.on_wait) > max_waits:
                waits = list(si.on_wait)
                si.on_wait = waits[-max_waits:]
                for w in waits[:-max_waits]:
                    nop = mybir.InstNoOp(
                        name=nc.get_next_instruction_name(), ins=[], outs=[]
                    )
                    nop.engine = inst.engine
                    nop.sync_info = mybir.SyncInfo(on_wait=[w], on_update=[])
                    nc.register_instruction(nop)
                    out.append(nop)
            out.append(inst)
        blk.instructions[:] = out


# ---------------------------------------------------------------------------

B, S, D = 2, 2048, 1024
H, HD = 16, 64
SCALE = 1.0 / HD
NCORES = 8
HPC = 4          # heads per core
DHC = HPC * HD   # 256 head-dims per core
P = 128
KC = D // P      # 8 contraction chunks for projections
SC = S // P      # 16 seq chunks of 128
QB = 512         # seq_q block for PV / O-proj
NQB = S // QB    # 4
EB = 1024        # exp/activation block (2 psum banks)

F32 = mybir.dt.float32
F32R = mybir.dt.float32r
BF16 = mybir.dt.bfloat16

_COMPILED = None


def _build_nc(loop_iters=None, phases="abc"):
    nc = bass.Bass("TRN2", target_bir_lowering=False, debug=False,
                   num_devices=NCORES)

    xT = nc.declare_dram_parameter("xT", [D, S], BF16, isOutput=False)
    wq = nc.declare_dram_parameter("wq", [D, DHC], BF16, isOutput=False)
    wk = nc.declare_dram_parameter("wk", [D, DHC], BF16, isOutput=False)
    wv = nc.declare_dram_parameter("wv", [D, DHC], BF16, isOutput=False)
    wo = nc.declare_dram_parameter("wo", [DHC, D], BF16, isOutput=False)
    bq = nc.declare_dram_parameter("bq", [2, P, 1], F32, isOutput=False)
    bk = nc.declare_dram_parameter("bk", [2, P, 1], F32, isOutput=False)
    bv = nc.declare_dram_parameter("bv", [P, DHC], F32, isOutput=False)
    tri = nc.declare_dram_parameter("tri", [P, P], BF16, isOutput=False)
    outT = nc.declare_dram_parameter("outT", [D, S], F32, isOutput=True)

    from contextlib import ExitStack
    with tile.TileContext(nc) as tc:
        _loop = ExitStack()
        if loop_iters:
            _loop.enter_context(tc.For_i(0, loop_iters, 1))

        # All DMA descriptor issues go on the Sync queue: it is otherwise
        # idle, while Scalar must stay free for the exp stream.
        def dma(out_ap, in_ap):
            return nc.sync.dma_start(out_ap, in_ap)

        with (
            tc.tile_pool(name="persist", bufs=1) as pp,
            tc.tile_pool(name="stage", bufs=2) as stage,
            tc.tile_pool(name="epool", bufs=6) as epool,
            tc.tile_pool(name="small", bufs=4) as small,
        ):
            # ---------------- Phase A: load, cast, project ----------------
            # persistent bf16 tensors
            xTb = [pp.tile([P, S], BF16, tag=f"xtb{k}", name=f"xtb{k}") for k in range(KC)]
            wqb = [pp.tile([P, DHC], BF16, tag=f"wqb{k}", name=f"wqb{k}") for k in range(KC)]
            wkb = [pp.tile([P, DHC], BF16, tag=f"wkb{k}", name=f"wkb{k}") for k in range(KC)]
            wvb = [pp.tile([P, DHC], BF16, tag=f"wvb{k}", name=f"wvb{k}") for k in range(KC)]
            wob = [pp.tile([P, D], BF16, tag=f"wob{c}", name=f"wob{c}") for c in range(2)]
            QT = [pp.tile([P, S], BF16, tag=f"qt{m}", name=f"qt{m}") for m in range(2)]
            KT = [pp.tile([P, S], BF16, tag=f"kt{m}", name=f"kt{m}") for m in range(2)]
            # V with a ones column per head: [h0(64) 1 | h1(64) 1 | ...]
            Vb = [pp.tile([P, HPC * 65], BF16, tag=f"vb{s}", name=f"vb{s}") for s in range(SC)]
            OT = [pp.tile([P, S], BF16, tag=f"ot{m}", name=f"ot{m}") for m in range(2)]
            trib = pp.tile([P, P], BF16, tag="trib")
            bq_sb = pp.tile([P, 2], F32, tag="bq")
            bk_sb = pp.tile([P, 2], F32, tag="bk")
            bv_bc = pp.tile([P, DHC], F32, tag="bvbc")
            ones1f = pp.tile([65, 64], F32, tag="ones1f")
            ones1 = pp.tile([65, 64], F32R, tag="ones1")

            # inputs arrive pre-cast to bf16: DMA straight into place,
            # interleaved per k-chunk so the projection k-loop starts early
            for k in range(KC):
                for (wsrc, wdst) in ((wq, wqb), (wk, wkb), (wv, wvb)):
                    dma(wdst[k][:], wsrc[k * P:(k + 1) * P, :])
                for hlf in range(2):
                    sl = slice(hlf * (S // 2), (hlf + 1) * (S // 2))
                    dma(xTb[k][:, sl], xT[k * P:(k + 1) * P, sl])
            for c in range(2):
                dma(wob[c][:], wo[c * P:(c + 1) * P, :])
            dma(trib[:], tri[:, :])

            dma(bq_sb[:, 0:1], bq[0])
            dma(bq_sb[:, 1:2], bq[1])
            dma(bk_sb[:, 0:1], bk[0])
            dma(bk_sb[:, 1:2], bk[1])
            dma(bv_bc[:], bv[:, :])
            nc.gpsimd.memset(ones1f[:], 1.0)
            with nc.allow_low_precision(reason="f32r ones for broadcast matmul"):
                nc.vector.tensor_copy(ones1[:], ones1f[:])

            with tc.tile_pool(name="apsum", bufs=6, space="PSUM") as aps:
                # QT / KT: out [dh-chunk 128, seq] ; lhsT = w chunk, rhs = xT
                def emit_qk_proj(wb, dst, bias, m):
                    for nb in range(NQB):
                        ps = aps.tile([P, QB], F32, tag="proj")
                        for k in range(KC):
                            nc.tensor.matmul(
                                ps[:],
                                wb[k][:, m * P:(m + 1) * P],
                                xTb[k][:, nb * QB:(nb + 1) * QB],
                                start=(k == 0),
                                stop=(k == KC - 1),
                            )
                        nc.vector.tensor_scalar_add(
                            dst[m][:, nb * QB:(nb + 1) * QB],
                            ps[:],
                            bias[:, m:m + 1],
                        )

                def emit_v_proj():
                    # V: out [seq chunk, 256]; lhsT = xT chunk, rhs = wv
                    for s in range(SC):
                        ps = aps.tile([P, DHC], F32, tag="proj")
                        for k in range(KC):
                            nc.tensor.matmul(
                                ps[:],
                                xTb[k][:, s * P:(s + 1) * P],
                                wvb[k][:],
                                start=(k == 0),
                                stop=(k == KC - 1),
                            )
                        # scatter heads into 65-stride layout with bias add
                        vout = Vb[s][:].rearrange("p (h x) -> p h x", h=HPC)[:, :, 0:64]
                        psr = ps[:].rearrange("p (h x) -> p h x", h=HPC)
                        bvr = bv_bc[:].rearrange("p (h x) -> p h x", h=HPC)
                        nc.vector.tensor_add(vout, psr, bvr)
                        ones = Vb[s][:].rearrange("p (h x) -> p h x", h=HPC)[:, :, 64:65]
                        nc.gpsimd.memset(ones, 1.0)

                emit_qk_proj(wqb, QT, bq_sb, 0)
                emit_qk_proj(wkb, KT, bk_sb, 0)
                emit_v_proj()
                emit_qk_proj(wqb, QT, bq_sb, 1)
                emit_qk_proj(wkb, KT, bk_sb, 1)

            # ---------------- Phase B: attention per head ----------------
            if "b" in phases:
             with (
                tc.tile_pool(name="scpsum", bufs=2, space="PSUM") as scp,
                tc.tile_pool(name="opsum", bufs=1, space="PSUM") as opp,
            ):
                pending_norm = [None]

                def flush_norm():
                    if pending_norm[0] is not None:
                        pending_norm[0]()
                        pending_norm[0] = None

                for h in range(HPC):
                    m, poff = h // 2, 64 * (h % 2)
                    kt_h = KT[m][poff:poff + 64, :]
                    qt_h = QT[m][poff:poff + 64, :]
                    ops = opp.tile([65, S], F32, tag="oacc", name="oacc")
                    for jc in range(SC):
                        W = P * (jc + 1)
                        e = epool.tile([P, S], BF16, tag="e")
                        # zero-fill the tail up to the next 512 boundary
                        # first (read by PV, must be 0; no dep on exp)
                        zf = (QB - W % QB) % QB
                        if zf:
                            nc.gpsimd.memset(e[:, W:W + zf], 0.0)
                        # scores S^T[jc] = K_h^T[:,jc-chunk]^T . Q_h^T in
                        # 1024-col psum tiles (2 banks); matmuls fill them
                        # in 512-col (bank) slices, one exp per tile.
                        for b0 in range(0, W, EB):
                            bw = min(EB, W - b0)
                            ps = scp.tile([P, EB], F32, tag="sc")
                            for c0 in range(0, bw, QB):
                                cw = min(QB, bw - c0)
                                nc.tensor.matmul(
                                    ps[:, c0:c0 + cw],
                                    kt_h[:, jc * P:(jc + 1) * P],
                                    qt_h[:, b0 + c0:b0 + c0 + cw],
                                    start=True,
                                    stop=True,
                                )
                            nc.scalar.activation(
                                e[:, b0:b0 + bw],
                                ps[:, 0:bw],
                                mybir.ActivationFunctionType.Exp,
                                scale=SCALE,
                            )
                        # mask the diagonal 128-block post-exp (x0/1)
                        nc.gpsimd.tensor_mul(
                            e[:, W - P:W], e[:, W - P:W], trib[:]
                        )
                        # PV: accumulate O'^T[qb] over jc
                        for qb in range((jc // 4) + 1):
                            nc.tensor.matmul(
                                ops[:, qb * QB:(qb + 1) * QB],
                                Vb[jc][:, 65 * h:65 * h + 65],
                                e[:, qb * QB:(qb + 1) * QB],
                                start=(jc == 4 * qb),
                                stop=(jc == SC - 1),
                            )
                    # evict O' to SBUF in one copy (releases all 4 psum
                    # banks for the next head's PV), defer the normalize
                    # emission so the next head's PE stream isn't blocked
                    # behind the eviction/normalize chain
                    o_sb = small.tile([65, S], F32R, tag="osb", bufs=2)
                    with nc.allow_low_precision(reason="f32r O' staging for broadcast matmul"):
                        nc.vector.tensor_copy(o_sb[:], ops[:])

                    def norm(m=m, poff=poff, o_sb=o_sb):
                        for qb in range(NQB):
                            # broadcast denom across partitions via K=1
                            # matmul, then reciprocal with all 64 lanes busy
                            dbp = scp.tile([64, QB], F32, tag="sc")
                            nc.tensor.matmul(
                                dbp[:], ones1[64:65, :],
                                o_sb[64:65, qb * QB:(qb + 1) * QB],
                                start=True, stop=True,
                            )
                            rrec = small.tile([64, QB], F32, tag="rrec", bufs=2)
                            nc.vector.reciprocal(rrec[:], dbp[:])
                            with nc.allow_low_precision(reason="f32r O' numerators"):
                                nc.vector.tensor_mul(
                                    OT[m][poff:poff + 64, qb * QB:(qb + 1) * QB],
                                    o_sb[0:64, qb * QB:(qb + 1) * QB],
                                    rrec[:],
                                )

                    flush_norm()
                    pending_norm[0] = norm
                flush_norm()

            # ---------------- Phase C: output projection ----------------
            if "c" in phases:
             with tc.tile_pool(name="cpsum", bufs=4, space="PSUM") as cps:
                for mo in range(D // P):
                    ot = stage.tile([P, S], F32, tag="outstage")
                    for qb in range(NQB):
                        ps = cps.tile([P, QB], F32, tag="oproj")
                        for c in range(2):
                            nc.tensor.matmul(
                                ps[:],
                                wob[c][:, mo * P:(mo + 1) * P],
                                OT[c][:, qb * QB:(qb + 1) * QB],
                                start=(c == 0),
                                stop=(c == 1),
                            )
                        nc.vector.tensor_copy(
                            ot[:, qb * QB:(qb + 1) * QB], ps[:])
                    dma(outT[mo * P:(mo + 1) * P, :], ot[:])
        _loop.close()
    _legalize_waits(nc)
    return nc


def _get_nc():
    global _COMPILED
    if _COMPILED is None:
        _COMPILED = _build_nc()
    return _COMPILED


def _make_in_maps(x, wq, bq, wk, bk, wv, bv, wo, bo):
    import ml_dtypes
    bf16 = ml_dtypes.bfloat16
    tri = np.tril(np.ones((P, P), dtype=bf16))
    in_maps = []
    for c in range(NCORES):
        b, g = c // NCORES * 0 + c // 4, c % 4
        cols = slice(DHC * g, DHC * (g + 1))
        in_maps.append({
            "xT": np.ascontiguousarray(x[b].T).astype(bf16),
            "wq": np.ascontiguousarray(wq[:, cols]).astype(bf16),
            "wk": np.ascontiguousarray(wk[:, cols]).astype(bf16),
            "wv": np.ascontiguousarray(wv[:, cols]).astype(bf16),
            "wo": np.ascontiguousarray(wo[cols, :]).astype(bf16),
            "bq": np.ascontiguousarray(bq[cols]).reshape(2, P, 1),
            "bk": np.ascontiguousarray(bk[cols]).reshape(2, P, 1),
            "bv": np.ascontiguousarray(np.broadcast_to(bv[cols].reshape(1, DHC), (P, DHC))),
            "tri": tri,
        })
    return in_maps


def kernel(x, wq, bq, wk, bk, wv, bv, wo, bo, _trace=False, _trace_kwargs=None):
    x = np.asarray(x, dtype=np.float32)
    assert x.shape == (B, S, D), x.shape
    nc = _get_nc()
    in_maps = _make_in_maps(
        x, np.asarray(wq), np.asarray(bq), np.asarray(wk), np.asarray(bk),
        np.asarray(wv), np.asarray(bv), np.asarray(wo), np.asarray(bo))
    kw = {}
    if _trace:
        kw = dict(trace=True, **(_trace_kwargs or {}))
    res = run_bass_kernel_spmd(nc, in_maps, list(range(NCORES)), **kw)
    out = np.empty((B, S, D), dtype=np.float32)
    for b in range(B):
        acc = np.zeros((D, S), dtype=np.float64)
        for g in range(4):
            acc += res.results[4 * b + g]["outT"]
        out[b] = acc.T.astype(np.float32) + np.asarray(bo, dtype=np.float32)
    kernel.last_result = res
    return out
